# revision 4
# baseline (speedup 1.0000x reference)
"""ConvLSTM block (B=16, T=16, 32->64ch, 64x64, 3x3 SAME conv) on 8 TRN2 cores.

Strategy: data-parallel over batch (2 images/core). All conv operands are
fp16 (moving-operand rate on the PE is 1 row/cycle, same as f32r, at half
the bytes; end-to-end rel err ~7e-4). The 3x3 conv over concat([x_t, h])
is computed as 7 K-packed matmuls per output tile instead of 9 K=96 ones:
shifted copies of the input planes are partition-stacked so each matmul
contracts over up to 128 rows:

  S1-S3 (ky=0..2): Buf1 = [x | x@(0,1) | h], window (y0+ky, 0)
                   -> taps x(ky,0), x(ky,1), h(ky,0)
  S4-S6 (ky=0..2): Buf2 = [h | h@(0,1)], window (y0+ky, 1)
                   -> taps h(ky,1), h(ky,2)
  S7:              Buf3 = [x@r0 | x@r1 | x@r2], window (y0, 2)
                   -> taps x(0,2), x(1,2), x(2,2)

Gate math: chunkA = [i;f] (one full-width sigmoid), chunkB = [g;o'] where
o' = tanh(o_raw/2) (weights/bias pre-halved) so one full-width tanh covers
both; sigma(z) = 0.5*tanh(z/2)+0.5 is recovered by writing v = 2h =
(o'+1)*tanh(c) into the h-planes with all h-columns of W pre-halved.
h-plane writes go directly from the compute engines (DVE + GPSIMD) into
the next step's conv buffers; the h@(0,0) copy for Buf2 is one contiguous
row-range DMA. Step 0 skips the h-only streams S4-S6 (h == 0).
"""

from contextlib import ExitStack

import numpy as np

import concourse.mybir as mybir
import concourse.tile as tile
from concourse import bacc
from concourse.bass_utils import run_bass_kernel_spmd

F32 = mybir.dt.float32
F16 = mybir.dt.float16
AF = mybir.ActivationFunctionType
ALU = mybir.AluOpType

# Problem shapes (hardcoded per harness contract).
B, T, CIN, HID, H, W = 16, 16, 32, 64, 64, 64
NCORES = 8
BL = B // NCORES            # images per core
CH = CIN + HID              # conv input channels
PH, PW = H + 2, W + 2       # zero-padded plane
RG_ROWS = 8                 # output rows per PSUM tile (8*64 = 512 = one bank)
NRG = H // RG_ROWS
NSTREAM = 7


def _build(steps=T):
    nc = bacc.Bacc("TRN2", target_bir_lowering=False, debug=False)
    x_d = nc.dram_tensor("xin", [BL, T, CIN, H, W], F32, kind="ExternalInput")
    w_d = nc.dram_tensor("win", [128, NSTREAM * 2 * 128], F16, kind="ExternalInput")
    b_d = nc.dram_tensor("bin", [128, 2], F32, kind="ExternalInput")
    o_d = nc.dram_tensor("out", [BL, HID, H, W], F32, kind="ExternalOutput")

    with tile.TileContext(nc) as tc:
        with ExitStack() as ctx:
            const = ctx.enter_context(tc.tile_pool(name="const", bufs=1))
            psum = ctx.enter_context(tc.tile_pool(name="psum", bufs=4, space="PSUM"))
            gp = ctx.enter_context(tc.tile_pool(name="gates", bufs=3))

            wsb = const.tile([128, NSTREAM * 2 * 128], F16, tag="wsb")
            nc.sync.dma_start(out=wsb[:, :], in_=w_d[:, :])
            bsb = const.tile([128, 2], F32, tag="bsb")
            nc.sync.dma_start(out=bsb[:, :], in_=b_d[:, :])

            # Ping-pong fp16 plane sets. buf1: x@(0,0) in [0:32), x@(0,1) in
            # [32:64), h@(0,0) in [64:128). buf2: h@(0,0) in [0:64),
            # h@(0,1) in [64:128). buf3: x row-shifted by 0/1/2.
            buf1 = [const.tile([128, BL, PH, PW], F16, tag=f"b1_{i}", name=f"b1_{i}")
                    for i in range(2)]
            buf2 = [const.tile([128, BL, PH, PW], F16, tag=f"b2_{i}", name=f"b2_{i}")
                    for i in range(2)]
            buf3 = [const.tile([96, BL, PH, PW], F16, tag=f"b3_{i}", name=f"b3_{i}")
                    for i in range(2)]
            for pb in buf1 + buf2 + buf3:
                nc.gpsimd.memset(pb[:, :, :, :], 0.0)
            # Cell state in partitions [64,128) (lane-aligned with f/o').
            cst = const.tile([128, BL, H * W], F16, tag="cst")
            nc.vector.memset(cst[:, :, :], 0.0)
            # f32 staging for x_t (GPSIMD converts to fp16 while scattering).
            xstage = const.tile([CIN, BL, H, W], F32, tag="xstage")

            for t in range(steps):
                cur1, nxt1 = buf1[t % 2], buf1[(t + 1) % 2]
                cur2, nxt2 = buf2[t % 2], buf2[(t + 1) % 2]
                cur3 = buf3[t % 2]

                # Stage x_t: HBM f32 -> fp16 padded plane + shifted copies.
                for img in range(BL):
                    nc.sync.dma_start(
                        out=xstage[:, img, :, :], in_=x_d[img, t, :, :, :]
                    )
                nc.gpsimd.tensor_copy(
                    cur1[0:CIN, :, 1 : H + 1, 1 : W + 1], xstage[:, :, :, :]
                )
                # x@(0,1): P2x[:, c] = P[:, c+1]
                nc.sync.dma_start(
                    out=cur1[32:64, :, :, 0 : PW - 1], in_=cur1[0:32, :, :, 1:PW]
                )
                # x row-shifted: grp_k rows [0 : PH-k) = P rows [k : PH)
                for k in range(3):
                    nc.sync.dma_start(
                        out=cur3[32 * k : 32 * k + 32, :, 0 : PH - k, :],
                        in_=cur1[0:32, :, k:PH, :],
                    )

                streams = list(range(NSTREAM)) if t > 0 else [0, 1, 2, 6]
                for img in range(BL):
                    for rg in range(NRG):
                        y0 = rg * RG_ROWS
                        ps = [
                            psum.tile([128, RG_ROWS, 64], F32, tag=f"ps{c}", name=f"ps{c}")
                            for c in range(2)
                        ]
                        for c in range(2):
                            for si, s in enumerate(streams):
                                if s < 3:
                                    k_sz = 128
                                    rhs = cur1[0:128, img, y0 + s : y0 + s + RG_ROWS, 0:64]
                                elif s < 6:
                                    k_sz = 128
                                    ky = s - 3
                                    rhs = cur2[0:128, img, y0 + ky : y0 + ky + RG_ROWS, 1:65]
                                else:
                                    k_sz = 96
                                    rhs = cur3[0:96, img, y0 : y0 + RG_ROWS, 2:66]
                                nc.tensor.matmul(
                                    out=ps[c][:, :, :],
                                    lhsT=wsb[0:k_sz, (s * 2 + c) * 128 : (s * 2 + c + 1) * 128],
                                    rhs=rhs,
                                    start=(si == 0),
                                    stop=(si == len(streams) - 1),
                                )

                        csl = cst[64:128, img, y0 * 64 : (y0 + RG_ROWS) * 64]

                        # chunkA = [i; f]: one full-width sigmoid.
                        sif = gp.tile([128, RG_ROWS, 64], F16, tag="sif")
                        nc.scalar.activation(
                            out=sif[:, :, :], in_=ps[0][:, :, :],
                            func=AF.Sigmoid, bias=bsb[:, 0:1],
                        )
                        # chunkB = [g; o']: one full-width tanh.
                        tgo = gp.tile([128, RG_ROWS, 64], F16, tag="tgo")
                        nc.scalar.activation(
                            out=tgo[:, :, :], in_=ps[1][:, :, :],
                            func=AF.Tanh, bias=bsb[:, 1:2],
                        )

                        # c = f*c + i*g with one cross-half DMA bridge.
                        pr1 = gp.tile([128, RG_ROWS, 64], F16, tag="pr1")
                        nc.vector.tensor_mul(pr1[0:64], sif[0:64], tgo[0:64])
                        nc.sync.dma_start(out=pr1[64:128], in_=pr1[0:64])
                        tmp = gp.tile([128, RG_ROWS, 64], F16, tag="tmp")
                        nc.vector.tensor_mul(tmp[64:128], sif[64:128], csl)
                        nc.vector.tensor_add(csl, tmp[64:128], pr1[64:128])
                        tct = gp.tile([128, RG_ROWS, 64], F16, tag="tct")
                        nc.scalar.activation(tct[64:128], csl, func=AF.Tanh)

                        if t < steps - 1:
                            # v = 2h = (o'+1)*tanh(c), written straight into
                            # the next step's conv planes (h-cols of W are
                            # pre-halved to compensate).
                            nc.vector.scalar_tensor_tensor(
                                out=nxt1[64:128, img, y0 + 1 : y0 + 9, 1 : W + 1],
                                in0=tgo[64:128], scalar=1.0, in1=tct[64:128],
                                op0=ALU.add, op1=ALU.mult,
                            )
                            nc.gpsimd.tensor_copy(
                                nxt2[64:128, img, y0 + 1 : y0 + 9, 0:W],
                                nxt1[64:128, img, y0 + 1 : y0 + 9, 1 : W + 1],
                            )
                            # h@(0,0) copy for buf2: contiguous row range.
                            nc.sync.dma_start(
                                out=nxt2[0:64, img, y0 + 1 : y0 + 9, :],
                                in_=nxt1[64:128, img, y0 + 1 : y0 + 9, :],
                            )
                        else:
                            # h = (0.5*o' + 0.5)*tanh(c); out = max(h, 0.01h)
                            e1 = gp.tile([128, RG_ROWS, 64], F32, tag="e1")
                            nc.vector.scalar_tensor_tensor(
                                out=e1[64:128], in0=tgo[64:128], scalar=0.5,
                                in1=tct[64:128], op0=ALU.mult, op1=ALU.mult,
                            )
                            ht = gp.tile([128, RG_ROWS, 64], F32, tag="ht")
                            nc.vector.scalar_tensor_tensor(
                                out=ht[64:128], in0=tct[64:128], scalar=0.5,
                                in1=e1[64:128], op0=ALU.mult, op1=ALU.add,
                            )
                            ost = gp.tile([128, RG_ROWS, 64], F32, tag="ost")
                            nc.vector.scalar_tensor_tensor(
                                out=ost[64:128], in0=ht[64:128], scalar=0.01,
                                in1=ht[64:128], op0=ALU.mult, op1=ALU.max,
                            )
                            nc.sync.dma_start(
                                out=o_d[img, :, y0 : y0 + RG_ROWS, :],
                                in_=ost[64:128, :, :],
                            )
    nc.compile()
    return nc


def _prep_weights(Wf, bf):
    Wp = np.asarray(Wf, np.float32).copy()     # [256, CH, 3, 3], gates [i,f,o,g]
    bp = np.asarray(bf, np.float32).copy()
    # h is fed as v = 2h: halve all h-columns.
    Wp[:, CIN:CH] *= 0.5
    # o' = tanh(o_raw/2): halve the o-gate rows and bias.
    Wp[128:192] *= 0.5
    bp = bp.copy()
    bp[128:192] *= 0.5
    # chunkA = [i; f], chunkB = [g; o'].
    chA = Wp[0:128]
    chB = np.concatenate([Wp[192:256], Wp[128:192]], axis=0)
    bA = bp[0:128]
    bB = np.concatenate([bp[192:256], bp[128:192]], axis=0)

    # wl[k, s, c, m]: stream s, chunk c, stationary column m, contraction k.
    wl = np.zeros((128, NSTREAM, 2, 128), np.float32)
    for c, Wc in enumerate([chA, chB]):
        for ky in range(3):                     # S1-S3: x(ky,0), x(ky,1), h(ky,0)
            wl[0:32, ky, c] = Wc[:, 0:CIN, ky, 0].T
            wl[32:64, ky, c] = Wc[:, 0:CIN, ky, 1].T
            wl[64:128, ky, c] = Wc[:, CIN:CH, ky, 0].T
        for ky in range(3):                     # S4-S6: h(ky,1), h(ky,2)
            wl[0:64, 3 + ky, c] = Wc[:, CIN:CH, ky, 1].T
            wl[64:128, 3 + ky, c] = Wc[:, CIN:CH, ky, 2].T
        for k in range(3):                      # S7: x(k,2)
            wl[32 * k : 32 * k + 32, 6, c] = Wc[:, 0:CIN, k, 2].T
    wl = np.ascontiguousarray(
        wl.reshape(128, NSTREAM * 2 * 128), dtype=np.float16
    )
    b2 = np.ascontiguousarray(np.stack([bA, bB], axis=1))  # [128, 2] f32
    return wl, b2


_NC_CACHE = {}


def _get_nc():
    if "nc" not in _NC_CACHE:
        _NC_CACHE["nc"] = _build()
    return _NC_CACHE["nc"]


def _in_maps(x, Wf, bf):
    x = np.ascontiguousarray(np.asarray(x, np.float32))
    wl, b2 = _prep_weights(Wf, bf)
    return [
        {
            "xin": np.ascontiguousarray(x[i * BL : (i + 1) * BL]),
            "win": wl,
            "bin": b2,
        }
        for i in range(NCORES)
    ]


def _run(x, W, b, trace=False, **spmd_kwargs):
    nc = _get_nc()
    res = run_bass_kernel_spmd(
        nc, _in_maps(x, W, b), core_ids=list(range(NCORES)), trace=trace,
        **spmd_kwargs,
    )
    out = np.concatenate([res.results[i]["out"] for i in range(NCORES)], axis=0)
    return np.ascontiguousarray(out, dtype=np.float32), res


def kernel(x, W, b):
    out, _ = _run(x, W, b)
    return out
